# revision 2
# baseline (speedup 1.0000x reference)
"""Trainium2 Bass kernel for CascadeClassifierGNN — pull-mode rewrite.

Dst-sharded pull design: nodes are partitioned contiguously across the 8
cores; each core owns the edges whose DESTINATION lies in its shard. Per
layer every core computes u = dinv * (h @ W') for its local nodes and the
full u table is AllGathered to every core's HBM. Each core then pulls its
in-edges' source rows with dma_gather (edge-major output) and segment-sums
them by destination with one-hot matmuls accumulated in PSUM: edges are
pre-sorted by (src-quarter, dst-tile) on the host, every 128-edge chunk
belongs to one destination tile, and the one-hot [128 edge x 128 slot]
matrices are generated on the vector engine 8 chunks at a time with a
single is_equal against a broadcast destination-slot stream. Host-side
node relabeling balances per-(quarter, tile) edge counts across cores so
one SPMD program (max-padded chunk counts) serves all 8 cores. BatchNorm
is folded into the layer weights on the host. Global mean pool uses
per-tile one-hot matmuls + a tiny AllReduce; the MLP head is replicated.
"""

import math
import os

import numpy as np

import concourse.bacc as bacc
import concourse.mybir as mybir
import concourse.tile as tile
from concourse import bass_utils
from concourse import library_config
from concourse.masks import make_identity

F32 = mybir.dt.float32
I32 = mybir.dt.int32
I16 = mybir.dt.int16
ALU = mybir.AluOpType

CFG = dict(N=100000, E=1600000, F_IN=10, H=64, B=128, C=3, EPS=1e-5)
M = 8            # cores
P = 128          # partitions
T = 104          # dst tiles per core
GT = 8           # tiles per super-group
NG = T // GT     # groups
NSP = T * P      # padded nodes per shard (13312)
QN = 2 * NSP     # rows per src quarter (26624, int16-addressable)
NQ = 4           # src quarters

TRACE = os.environ.get("GNN_TRACE", "0") == "1"
SINGLE_PACKET = os.environ.get("GNN_SP", "0") == "1"
LAST_EXEC_NS = None


def _fold_bn(Wl, bl, gl, bel, ml, vl, eps):
    A = (np.asarray(gl, np.float32)
         / np.sqrt(np.asarray(vl, np.float32) + np.float32(eps)))
    Wp = (np.asarray(Wl, np.float32) * A[None, :]).astype(np.float32)
    Bv = ((np.asarray(bl, np.float32) - np.asarray(ml, np.float32)) * A
          + np.asarray(bel, np.float32)).astype(np.float32)
    return Wp, Bv


def _balance_tiles(vq):
    """Assign NS nodes (rows of vq [NS, 4]) to T tiles of <=128 slots,
    balancing per-quarter loads. Returns local slot index per node."""
    NS = vq.shape[0]
    tot = vq.sum(axis=1)
    order = np.argsort(-tot, kind="stable")
    loads = np.zeros((T, NQ), np.int64)
    counts = np.zeros(T, np.int64)
    slot = np.empty(NS, np.int64)
    for i in order:
        v = vq[i]
        score = (loads + v[None, :]).max(axis=1).astype(np.float64)
        score[counts >= P] = np.inf
        t = int(np.argmin(score))
        slot[i] = t * P + counts[t]
        counts[t] += 1
        loads[t] += v
    return slot


def preprocess(x, edge_index, batch,
               W1, b1, g1, be1, m1, v1,
               W2, b2, g2, be2, m2, v2,
               W3, b3, g3, be3, m3, v3,
               fw1, fb1, fw2, fb2, cfg=CFG):
    N, E, F_IN, H, B, C = (cfg["N"], cfg["E"], cfg["F_IN"], cfg["H"],
                           cfg["B"], cfg["C"])
    NS = N // M
    x = np.asarray(x, dtype=np.float32)
    src = np.asarray(edge_index[0], dtype=np.int64)
    dst = np.asarray(edge_index[1], dtype=np.int64)
    batch = np.asarray(batch, dtype=np.int64)

    deg = (np.bincount(dst, minlength=N) + 1.0).astype(np.float32)
    dinv = (1.0 / np.sqrt(deg)).astype(np.float32)

    core_of = (np.arange(N) // NS).astype(np.int64)     # node -> owner
    e_q = (src // (2 * NS)).astype(np.int64)            # edge src quarter

    indeg_q = np.zeros((N, NQ), np.int64)
    np.add.at(indeg_q, (dst, e_q), 1)

    local_index = np.empty(N, np.int64)
    for c in range(M):
        lo = c * NS
        local_index[lo:lo + NS] = _balance_tiles(indeg_q[lo:lo + NS])

    row = core_of * NSP + local_index                   # node -> table row

    e_core = core_of[dst]
    e_t = local_index[dst] // P
    e_slot = local_index[dst] % P
    e_qrow = row[src] - e_q * QN                        # [0, QN)

    cnt = np.zeros((M, NQ, T), np.int64)
    np.add.at(cnt, (e_core, e_q, e_t), 1)
    k_qt = np.ceil(cnt / P).max(axis=0).astype(np.int64)   # [NQ, T]
    rows_qt = P * k_qt

    # stream offsets: q-major, t-minor
    flat = rows_qt.reshape(-1)
    off_flat = np.concatenate([[0], np.cumsum(flat)[:-1]])
    off_qt = off_flat.reshape(NQ, T)
    total_rows = int(flat.sum())
    assert total_rows % P == 0
    NCH = total_rows // P                               # chunks per layer
    NCH8 = ((NCH + 7) // 8) * 8

    # rank of each edge within its (core, q, t) bucket; sort by source row
    # within the bucket so each gather instruction walks the table forward
    key = (e_core * NQ + e_q) * T + e_t
    order = np.lexsort((e_qrow, key))
    ks = key[order]
    first = np.r_[True, ks[1:] != ks[:-1]]
    start = np.where(first, np.arange(E), 0)
    start = np.maximum.accumulate(start)
    rank = np.arange(E) - start

    pos = off_qt[e_q[order], e_t[order]] + rank
    c_sorted = e_core[order]

    gidx = np.zeros((M, total_rows), np.int16)
    dstv = np.full((M, total_rows), 300.0, np.float32)
    gidx[c_sorted, pos] = e_qrow[order].astype(np.int16)
    dstv[c_sorted, pos] = e_slot[order].astype(np.float32)

    # wrapped idx layout [16, total/16] replicated to 128 partitions
    gidx16 = gidx.reshape(M, total_rows // 16, 16).transpose(0, 2, 1)
    gidx128 = np.tile(gidx16, (1, 8, 1)).copy()
    # dstf [128, NCH8]: column c = dst slots of chunk c
    dstf = np.full((M, P, NCH8), 300.0, np.float32)
    dstf[:, :, :NCH] = dstv.reshape(M, NCH, P).transpose(0, 2, 1)

    # node-side shards (relabeled)
    xs = np.zeros((M, NSP, F_IN), np.float32)
    dinv_t = np.zeros((M, P, T), np.float32)
    batch_t = np.full((M, P, T), -1.0, np.float32)
    for c in range(M):
        lo = c * NS
        li = local_index[lo:lo + NS]
        xs[c, li] = x[lo:lo + NS]
        dv = np.zeros(NSP, np.float32)
        dv[li] = dinv[lo:lo + NS]
        dinv_t[c] = dv.reshape(T, P).T
        bt = np.full(NSP, -1.0, np.float32)
        bt[li] = batch[lo:lo + NS].astype(np.float32)
        batch_t[c] = bt.reshape(T, P).T

    counts = np.bincount(batch, minlength=B).astype(np.float32)
    cinv = (1.0 / np.maximum(counts, 1.0)).astype(np.float32)

    eps = cfg["EPS"]
    W1p, B1 = _fold_bn(W1, b1, g1, be1, m1, v1, eps)
    W2p, B2 = _fold_bn(W2, b2, g2, be2, m2, v2, eps)
    W3p, B3 = _fold_bn(W3, b3, g3, be3, m3, v3, eps)

    def bc(v, reps):
        return np.ascontiguousarray(
            np.tile(np.asarray(v, np.float32)[None, :], (P, reps)))

    iota8 = np.tile(np.arange(P, dtype=np.float32)[None, :], (P, 8))

    shared = {
        "W1p": W1p, "W2p": W2p, "W3p": W3p,
        "B1bc": bc(B1, GT), "B2bc": bc(B2, GT), "B3bc": bc(B3, GT),
        "fw1": np.asarray(fw1, np.float32), "fw2": np.asarray(fw2, np.float32),
        "fb1bc": bc(fb1, 1), "fb2bc": bc(fb2, 1),
        "cinv": cinv.reshape(B, 1), "iota8": iota8,
    }
    in_maps = []
    for c in range(M):
        im = {"x_sh": xs[c], "dinv_t": dinv_t[c], "batch_t": batch_t[c],
              "gidx": gidx128[c], "dstf": dstf[c]}
        im.update(shared)
        in_maps.append(im)

    meta = dict(cfg=tuple(sorted(cfg.items())),
                k_qt=tuple(map(tuple, k_qt)),
                total_rows=total_rows, NCH=NCH, NCH8=NCH8)
    return in_maps, meta


def cache_key(meta):
    return (meta["cfg"], meta["k_qt"])


def build_program(meta):
    cfg = dict(meta["cfg"])
    F_IN, H, B, C = cfg["F_IN"], cfg["H"], cfg["B"], cfg["C"]
    HB = H // 2
    k_qt = np.asarray(meta["k_qt"], np.int64)
    total_rows = meta["total_rows"]
    NCH8 = meta["NCH8"]
    IC = total_rows // 16

    # rows per (q, G) gather
    rows_qG = np.array([[int(P * k_qt[q, G * GT:(G + 1) * GT].sum())
                         for G in range(NG)] for q in range(NQ)])
    MAXR = int(rows_qG.max())

    nc = bacc.Bacc("TRN2", target_bir_lowering=False, debug=False,
                   num_devices=M)

    x_sh = nc.dram_tensor("x_sh", [NSP, F_IN], F32, kind="ExternalInput")
    dinv_t_d = nc.dram_tensor("dinv_t", [P, T], F32, kind="ExternalInput")
    batch_t_d = nc.dram_tensor("batch_t", [P, T], F32, kind="ExternalInput")
    gidx_d = nc.dram_tensor("gidx", [P, IC], I16, kind="ExternalInput")
    dstf_d = nc.dram_tensor("dstf", [P, NCH8], F32, kind="ExternalInput")
    W1p = nc.dram_tensor("W1p", [F_IN, H], F32, kind="ExternalInput")
    W2p = nc.dram_tensor("W2p", [H, H], F32, kind="ExternalInput")
    W3p = nc.dram_tensor("W3p", [H, H], F32, kind="ExternalInput")
    Bbc_d = [nc.dram_tensor(f"B{l}bc", [P, GT * H], F32, kind="ExternalInput")
             for l in (1, 2, 3)]
    fw1_d = nc.dram_tensor("fw1", [H, HB], F32, kind="ExternalInput")
    fw2_d = nc.dram_tensor("fw2", [HB, C], F32, kind="ExternalInput")
    fb1bc = nc.dram_tensor("fb1bc", [P, HB], F32, kind="ExternalInput")
    fb2bc = nc.dram_tensor("fb2bc", [P, C], F32, kind="ExternalInput")
    cinv_d = nc.dram_tensor("cinv", [B, 1], F32, kind="ExternalInput")
    iota8_d = nc.dram_tensor("iota8", [P, 8 * P], F32, kind="ExternalInput")
    out_d = nc.dram_tensor("out", [B, C], F32, kind="ExternalOutput")

    ag_in = nc.dram_tensor("ag_in", [NSP, H], F32)
    table = nc.dram_tensor("table", [M * NSP, H], F32)
    pool_in = nc.dram_tensor("pool_in", [B, H], F32)
    pool_out = nc.dram_tensor("pool_out", [B, H], F32)

    groups = [list(range(M))]

    def super_rows(dram, g, width):
        rows = dram[g * GT * P:(g + 1) * GT * P, :]
        return rows.rearrange("(t p) j -> p t j", p=P)

    with tile.TileContext(nc) as tc:
        with (
            tc.tile_pool(name="resident", bufs=1) as rp,
            tc.tile_pool(name="work", bufs=3) as wp,
            tc.tile_pool(name="oh", bufs=4) as ohp,
            tc.tile_pool(name="gather", bufs=3) as gp,
            tc.tile_pool(name="pseg", bufs=2, space="PSUM") as pseg,
            tc.tile_pool(name="ptr", bufs=1, space="PSUM") as ptr,
            tc.tile_pool(name="pu", bufs=1, space="PSUM") as pu,
            tc.tile_pool(name="pacc", bufs=1, space="PSUM") as pacc,
        ):
            ident = rp.tile([P, P], F32, tag="ident")
            make_identity(nc, ident[:])
            nc.gpsimd.load_library(library_config.mlp)

            iota8 = rp.tile([P, 8 * P], F32, tag="iota8")
            nc.sync.dma_start(out=iota8[:], in_=iota8_d[:, :])
            dstf = rp.tile([P, NCH8], F32, tag="dstf")
            nc.sync.dma_start(out=dstf[:], in_=dstf_d[:, :])
            gidx = rp.tile([P, IC], I16, tag="gidx")
            nc.sync.dma_start(out=gidx[:], in_=gidx_d[:, :])
            batchf = rp.tile([P, T], F32, tag="batchf")
            nc.sync.dma_start(out=batchf[:], in_=batch_t_d[:, :])

            ones_t = rp.tile([P, H], F32, tag="ones")
            nc.vector.memset(ones_t[:], 1.0)
            dinv_ts = rp.tile([P, T], F32, tag="dinv_ts")
            nc.sync.dma_start(out=dinv_ts[:], in_=dinv_t_d[:, :])
            dinvbc = rp.tile([P, T * H], F32, tag="dinvbc")
            for t in range(T):
                nc.vector.tensor_scalar(
                    dinvbc[:, t * H:(t + 1) * H], ones_t[:],
                    dinv_ts[:, t:t + 1], None, ALU.mult)

            uloc = rp.tile([P, T * H], F32, tag="uloc")
            acc = rp.tile([P, T * H], F32, tag="acc")

            w1s = rp.tile([F_IN, H], F32, tag="w1s")
            nc.sync.dma_start(out=w1s[:], in_=W1p[:, :])
            w2s = rp.tile([H, H], F32, tag="w2s")
            nc.sync.dma_start(out=w2s[:], in_=W2p[:, :])
            w3s = rp.tile([H, H], F32, tag="w3s")
            nc.sync.dma_start(out=w3s[:], in_=W3p[:, :])
            bbc = []
            for l in range(3):
                t_ = rp.tile([P, GT * H], F32, tag=f"bbc{l}")
                nc.sync.dma_start(out=t_[:], in_=Bbc_d[l][:, :])
                bbc.append(t_)
            fw1s = rp.tile([H, HB], F32, tag="fw1s")
            nc.sync.dma_start(out=fw1s[:], in_=fw1_d[:, :])
            fw2s = rp.tile([HB, C], F32, tag="fw2s")
            nc.sync.dma_start(out=fw2s[:], in_=fw2_d[:, :])
            fb1s = rp.tile([P, HB], F32, tag="fb1s")
            nc.sync.dma_start(out=fb1s[:], in_=fb1bc[:, :])
            fb2s = rp.tile([P, C], F32, tag="fb2s")
            nc.sync.dma_start(out=fb2s[:], in_=fb2bc[:, :])
            cinvs = rp.tile([B, 1], F32, tag="cinvs")
            nc.sync.dma_start(out=cinvs[:], in_=cinv_d[:, :])

            def dense_from(rt_ap, G, w_s, kdim):
                """u'[G] = dinv * (rt @ W'); write uloc[G] and ag_in[G]."""
                gslice = slice(G * GT * H, (G + 1) * GT * H)
                psT = ptr.tile([H, GT * P], F32, tag="psT")
                for t8 in range(GT):
                    nc.tensor.transpose(
                        out=psT[:kdim, t8 * P:(t8 + 1) * P],
                        in_=rt_ap[:, t8 * kdim:(t8 + 1) * kdim],
                        identity=ident[:])
                rtT = wp.tile([H, GT * P], F32, tag="rtT")
                nc.vector.tensor_copy(rtT[:kdim, :], psT[:kdim, :])
                psU = pu.tile([P, GT * H], F32, tag="psU")
                for t8 in range(GT):
                    nc.tensor.matmul(
                        out=psU[:, t8 * H:(t8 + 1) * H],
                        lhsT=rtT[:kdim, t8 * P:(t8 + 1) * P],
                        rhs=w_s[:], start=True, stop=True)
                nc.vector.tensor_tensor(
                    out=uloc[:, gslice], in0=psU[:],
                    in1=dinvbc[:, gslice], op=ALU.mult)
                nc.sync.dma_start(
                    out=super_rows(ag_in, G, H),
                    in_=uloc[:, gslice].rearrange("p (t j) -> p t j", j=H))

            # ---------------- layer-0 dense: u1 = dinv * (x @ W1') --------
            for G in range(NG):
                xt = wp.tile([P, GT * F_IN], F32, tag="xt")
                nc.sync.dma_start(
                    out=xt[:].rearrange("p (t j) -> p t j", j=F_IN),
                    in_=super_rows(x_sh, G, F_IN))
                dense_from(xt[:], G, w1s, F_IN)

            oh_state = {"tile": None, "c0": -1}

            def get_oh(c):
                c0 = (c // 8) * 8
                if oh_state["c0"] != c0:
                    oh8 = ohp.tile([P, 8 * P], F32, tag="oh8")
                    nc.vector.tensor_tensor(
                        out=oh8[:].rearrange("p (c j) -> p c j", j=P),
                        in0=iota8[:].rearrange("p (c j) -> p c j", j=P),
                        in1=dstf[:, c0:c0 + 8].to_broadcast([P, 8, P]),
                        op=ALU.is_equal)
                    oh_state["tile"] = oh8
                    oh_state["c0"] = c0
                return oh_state["tile"][:, (c - c0) * P:(c - c0 + 1) * P]

            # ---------------- layers ----------------
            for l in range(3):
                nc.gpsimd.collective_compute(
                    "AllGather", ALU.bypass, replica_groups=groups,
                    ins=[ag_in.ap().opt()], outs=[table.ap().opt()])

                goff = 0     # rows consumed so far (global chunk counter)
                for q in range(NQ):
                    for G in range(NG):
                        rows_G = int(rows_qG[q, G])
                        if rows_G == 0:
                            continue
                        buf = gp.tile([P, (MAXR // P) * H], F32, tag="buf")
                        nc.gpsimd.dma_gather(
                            buf[:, :(rows_G // P) * H].rearrange(
                                "p (c j) -> p c j", j=H),
                            table[q * QN:(q + 1) * QN, :],
                            gidx[:, goff // 16:(goff + rows_G) // 16],
                            rows_G, rows_G, H,
                            single_packet=SINGLE_PACKET)
                        seg = pseg.tile([P, GT * H], F32, tag="seg")
                        lc = 0
                        for t8 in range(GT):
                            t = G * GT + t8
                            k = int(k_qt[q, t])
                            for j in range(k):
                                c = goff // P + lc
                                oh = get_oh(c)
                                nc.tensor.matmul(
                                    out=seg[:, t8 * H:(t8 + 1) * H],
                                    lhsT=oh,
                                    rhs=buf[:, lc * H:(lc + 1) * H],
                                    start=(j == 0), stop=(j == k - 1))
                                lc += 1
                        gslice = slice(G * GT * H, (G + 1) * GT * H)
                        if q == 0:
                            nc.vector.tensor_copy(acc[:, gslice], seg[:])
                        else:
                            nc.vector.tensor_add(acc[:, gslice],
                                                 acc[:, gslice], seg[:])
                        goff += rows_G

                # tile phase: rt -> (dense | pool)
                if l == 2:
                    pool_ps = pacc.tile([B, H], F32, tag="pool_ps")
                for G in range(NG):
                    gslice = slice(G * GT * H, (G + 1) * GT * H)
                    rt = wp.tile([P, GT * H], F32, tag="rt")
                    nc.vector.tensor_add(rt[:], acc[:, gslice],
                                         uloc[:, gslice])
                    nc.vector.tensor_tensor(out=rt[:], in0=rt[:],
                                            in1=dinvbc[:, gslice],
                                            op=ALU.mult)
                    nc.vector.tensor_add(rt[:], rt[:], bbc[l][:])
                    nc.vector.tensor_scalar_max(rt[:], rt[:], 0.0)
                    if l < 2:
                        dense_from(rt[:], G, w2s if l == 0 else w3s, H)
                    else:
                        for t8 in range(GT):
                            t = G * GT + t8
                            S = wp.tile([P, P], F32, tag="S")
                            nc.vector.tensor_scalar(
                                S[:], iota8[:, :P], batchf[:, t:t + 1],
                                None, ALU.is_equal)
                            nc.tensor.matmul(
                                out=pool_ps[:], lhsT=S[:],
                                rhs=rt[:, t8 * H:(t8 + 1) * H],
                                start=(t == 0), stop=(t == T - 1))

            # ---------------- pool + MLP head ----------------
            pool_sb = wp.tile([B, H], F32, tag="pool_sb")
            nc.vector.tensor_copy(pool_sb[:], pool_ps[:])
            nc.sync.dma_start(out=pool_in[:, :], in_=pool_sb[:])
            nc.gpsimd.collective_compute(
                "AllReduce", ALU.add, replica_groups=groups,
                ins=[pool_in.ap().opt()], outs=[pool_out.ap().opt()])

            pooled = wp.tile([B, H], F32, tag="pooled")
            nc.sync.dma_start(out=pooled[:], in_=pool_out[:, :])
            nc.vector.tensor_scalar(pooled[:], pooled[:], cinvs[:], None,
                                    ALU.mult)

            trp = pu.tile([H, B], F32, tag="psU")
            nc.tensor.transpose(out=trp[:], in_=pooled[:], identity=ident[:])
            pT = wp.tile([H, B], F32, tag="pT")
            nc.vector.tensor_copy(pT[:], trp[:])
            h1ps = pu.tile([B, HB], F32, tag="psU")
            nc.tensor.matmul(out=h1ps[:], lhsT=pT[:], rhs=fw1s[:],
                             start=True, stop=True)
            h1 = wp.tile([B, HB], F32, tag="h1")
            nc.vector.tensor_add(h1[:], h1ps[:], fb1s[:])
            nc.vector.tensor_scalar_max(h1[:], h1[:], 0.0)
            tr2 = pu.tile([HB, B], F32, tag="psU")
            nc.tensor.transpose(out=tr2[:], in_=h1[:], identity=ident[:])
            h1T = wp.tile([HB, B], F32, tag="h1T")
            nc.vector.tensor_copy(h1T[:], tr2[:])
            o_ps = pu.tile([B, C], F32, tag="psU")
            nc.tensor.matmul(out=o_ps[:], lhsT=h1T[:], rhs=fw2s[:],
                             start=True, stop=True)
            o_sb = wp.tile([B, C], F32, tag="o_sb")
            nc.vector.tensor_add(o_sb[:], o_ps[:], fb2s[:])
            nc.sync.dma_start(out=out_d[:, :], in_=o_sb[:])

    nc.compile()
    return nc


_CACHE: dict = {}


def kernel(**inputs) -> np.ndarray:
    global LAST_EXEC_NS
    in_maps, meta = preprocess(**inputs)
    key = cache_key(meta)
    nc = _CACHE.get(key)
    if nc is None:
        nc = build_program(meta)
        _CACHE[key] = nc
    res = bass_utils.run_bass_kernel_spmd(
        nc, in_maps, core_ids=list(range(M)), trace=TRACE)
    LAST_EXEC_NS = res.exec_time_ns
    return np.asarray(res.results[0]["out"])


# revision 3
# speedup vs baseline: 1.0071x; 1.0071x over previous
"""Trainium2 Bass kernel for CascadeClassifierGNN — pull-mode rewrite.

Dst-sharded pull design: nodes are partitioned contiguously across the 8
cores; each core owns the edges whose DESTINATION lies in its shard. Per
layer every core computes u = dinv * (h @ W') for its local nodes and the
full u table is AllGathered to every core's HBM. Each core then pulls its
in-edges' source rows with dma_gather (edge-major output) and segment-sums
them by destination with one-hot matmuls accumulated in PSUM: edges are
pre-sorted by (src-quarter, dst-tile) on the host, every 128-edge chunk
belongs to one destination tile, and the one-hot [128 edge x 128 slot]
matrices are generated on the vector engine 8 chunks at a time with a
single is_equal against a broadcast destination-slot stream. Host-side
node relabeling balances per-(quarter, tile) edge counts across cores so
one SPMD program (max-padded chunk counts) serves all 8 cores. BatchNorm
is folded into the layer weights on the host. Global mean pool uses
per-tile one-hot matmuls + a tiny AllReduce; the MLP head is replicated.
"""

import math
import os

import numpy as np

import concourse.bacc as bacc
import concourse.mybir as mybir
import concourse.tile as tile
from concourse import bass_utils
from concourse import library_config
from concourse.masks import make_identity

F32 = mybir.dt.float32
I32 = mybir.dt.int32
I16 = mybir.dt.int16
ALU = mybir.AluOpType

CFG = dict(N=100000, E=1600000, F_IN=10, H=64, B=128, C=3, EPS=1e-5)
M = 8            # cores
P = 128          # partitions
T = 100          # dst tiles per core
GT = 5           # tiles per super-group
NG = T // GT     # groups
NSP = T * P      # padded nodes per shard (13312)
QN = 2 * NSP     # rows per src quarter (26624, int16-addressable)
NQ = 4           # src quarters

TRACE = os.environ.get("GNN_TRACE", "0") == "1"
SINGLE_PACKET = os.environ.get("GNN_SP", "0") == "1"
LAST_EXEC_NS = None


def _fold_bn(Wl, bl, gl, bel, ml, vl, eps):
    A = (np.asarray(gl, np.float32)
         / np.sqrt(np.asarray(vl, np.float32) + np.float32(eps)))
    Wp = (np.asarray(Wl, np.float32) * A[None, :]).astype(np.float32)
    Bv = ((np.asarray(bl, np.float32) - np.asarray(ml, np.float32)) * A
          + np.asarray(bel, np.float32)).astype(np.float32)
    return Wp, Bv


def _balance_tiles(vq):
    """Assign NS nodes (rows of vq [NS, 4]) to T tiles of <=128 slots,
    balancing per-quarter loads. Returns local slot index per node."""
    NS = vq.shape[0]
    tot = vq.sum(axis=1)
    order = np.argsort(-tot, kind="stable")
    loads = np.zeros((T, NQ), np.int64)
    counts = np.zeros(T, np.int64)
    slot = np.empty(NS, np.int64)
    for i in order:
        v = vq[i]
        score = (loads + v[None, :]).max(axis=1).astype(np.float64)
        score[counts >= P] = np.inf
        t = int(np.argmin(score))
        slot[i] = t * P + counts[t]
        counts[t] += 1
        loads[t] += v
    return slot


def preprocess(x, edge_index, batch,
               W1, b1, g1, be1, m1, v1,
               W2, b2, g2, be2, m2, v2,
               W3, b3, g3, be3, m3, v3,
               fw1, fb1, fw2, fb2, cfg=CFG):
    N, E, F_IN, H, B, C = (cfg["N"], cfg["E"], cfg["F_IN"], cfg["H"],
                           cfg["B"], cfg["C"])
    NS = N // M
    x = np.asarray(x, dtype=np.float32)
    src = np.asarray(edge_index[0], dtype=np.int64)
    dst = np.asarray(edge_index[1], dtype=np.int64)
    batch = np.asarray(batch, dtype=np.int64)

    deg = (np.bincount(dst, minlength=N) + 1.0).astype(np.float32)
    dinv = (1.0 / np.sqrt(deg)).astype(np.float32)

    core_of = (np.arange(N) // NS).astype(np.int64)     # node -> owner
    e_q = (src // (2 * NS)).astype(np.int64)            # edge src quarter

    indeg_q = np.zeros((N, NQ), np.int64)
    np.add.at(indeg_q, (dst, e_q), 1)

    local_index = np.empty(N, np.int64)
    for c in range(M):
        lo = c * NS
        local_index[lo:lo + NS] = _balance_tiles(indeg_q[lo:lo + NS])

    row = core_of * NSP + local_index                   # node -> table row

    e_core = core_of[dst]
    e_t = local_index[dst] // P
    e_slot = local_index[dst] % P
    e_qrow = row[src] - e_q * QN                        # [0, QN)

    cnt = np.zeros((M, NQ, T), np.int64)
    np.add.at(cnt, (e_core, e_q, e_t), 1)
    k_qt = np.ceil(cnt / P).max(axis=0).astype(np.int64)   # [NQ, T]
    rows_qt = P * k_qt

    # stream offsets: q-major, t-minor
    flat = rows_qt.reshape(-1)
    off_flat = np.concatenate([[0], np.cumsum(flat)[:-1]])
    off_qt = off_flat.reshape(NQ, T)
    total_rows = int(flat.sum())
    assert total_rows % P == 0
    NCH = total_rows // P                               # chunks per layer
    NCH8 = ((NCH + 7) // 8) * 8

    # rank of each edge within its (core, q, t) bucket; sort by source row
    # within the bucket so each gather instruction walks the table forward
    key = (e_core * NQ + e_q) * T + e_t
    order = np.lexsort((e_qrow, key))
    ks = key[order]
    first = np.r_[True, ks[1:] != ks[:-1]]
    start = np.where(first, np.arange(E), 0)
    start = np.maximum.accumulate(start)
    rank = np.arange(E) - start

    pos = off_qt[e_q[order], e_t[order]] + rank
    c_sorted = e_core[order]

    gidx = np.zeros((M, total_rows), np.int16)
    dstv = np.full((M, total_rows), 300.0, np.float32)
    gidx[c_sorted, pos] = e_qrow[order].astype(np.int16)
    dstv[c_sorted, pos] = e_slot[order].astype(np.float32)

    # wrapped idx layout [16, total/16] replicated to 128 partitions
    gidx16 = gidx.reshape(M, total_rows // 16, 16).transpose(0, 2, 1)
    gidx128 = np.tile(gidx16, (1, 8, 1)).copy()
    # dstf [128, NCH8]: column c = dst slots of chunk c
    dstf = np.full((M, P, NCH8), 300.0, np.float32)
    dstf[:, :, :NCH] = dstv.reshape(M, NCH, P).transpose(0, 2, 1)

    # node-side shards (relabeled)
    xs = np.zeros((M, NSP, F_IN), np.float32)
    dinv_t = np.zeros((M, P, T), np.float32)
    batch_t = np.full((M, P, T), -1.0, np.float32)
    for c in range(M):
        lo = c * NS
        li = local_index[lo:lo + NS]
        xs[c, li] = x[lo:lo + NS]
        dv = np.zeros(NSP, np.float32)
        dv[li] = dinv[lo:lo + NS]
        dinv_t[c] = dv.reshape(T, P).T
        bt = np.full(NSP, -1.0, np.float32)
        bt[li] = batch[lo:lo + NS].astype(np.float32)
        batch_t[c] = bt.reshape(T, P).T

    counts = np.bincount(batch, minlength=B).astype(np.float32)
    cinv = (1.0 / np.maximum(counts, 1.0)).astype(np.float32)

    eps = cfg["EPS"]
    W1p, B1 = _fold_bn(W1, b1, g1, be1, m1, v1, eps)
    W2p, B2 = _fold_bn(W2, b2, g2, be2, m2, v2, eps)
    W3p, B3 = _fold_bn(W3, b3, g3, be3, m3, v3, eps)

    def bc(v, reps):
        return np.ascontiguousarray(
            np.tile(np.asarray(v, np.float32)[None, :], (P, reps)))

    iota8 = np.tile(np.arange(P, dtype=np.float32)[None, :], (P, 8))

    shared = {
        "W1p": W1p, "W2p": W2p, "W3p": W3p,
        "B1bc": bc(B1, GT), "B2bc": bc(B2, GT), "B3bc": bc(B3, GT),
        "fw1": np.asarray(fw1, np.float32), "fw2": np.asarray(fw2, np.float32),
        "fb1bc": bc(fb1, 1), "fb2bc": bc(fb2, 1),
        "cinv": cinv.reshape(B, 1), "iota8": iota8,
    }
    in_maps = []
    for c in range(M):
        im = {"x_sh": xs[c], "dinv_t": dinv_t[c], "batch_t": batch_t[c],
              "gidx": gidx128[c], "dstf": dstf[c]}
        im.update(shared)
        in_maps.append(im)

    meta = dict(cfg=tuple(sorted(cfg.items())),
                k_qt=tuple(map(tuple, k_qt)),
                total_rows=total_rows, NCH=NCH, NCH8=NCH8)
    return in_maps, meta


def cache_key(meta):
    return (meta["cfg"], meta["k_qt"])


def build_program(meta):
    cfg = dict(meta["cfg"])
    F_IN, H, B, C = cfg["F_IN"], cfg["H"], cfg["B"], cfg["C"]
    HB = H // 2
    k_qt = np.asarray(meta["k_qt"], np.int64)
    total_rows = meta["total_rows"]
    NCH8 = meta["NCH8"]
    IC = total_rows // 16

    # rows per (q, G) gather
    rows_qG = np.array([[int(P * k_qt[q, G * GT:(G + 1) * GT].sum())
                         for G in range(NG)] for q in range(NQ)])
    MAXR = int(rows_qG.max())

    nc = bacc.Bacc("TRN2", target_bir_lowering=False, debug=False,
                   num_devices=M)

    x_sh = nc.dram_tensor("x_sh", [NSP, F_IN], F32, kind="ExternalInput")
    dinv_t_d = nc.dram_tensor("dinv_t", [P, T], F32, kind="ExternalInput")
    batch_t_d = nc.dram_tensor("batch_t", [P, T], F32, kind="ExternalInput")
    gidx_d = nc.dram_tensor("gidx", [P, IC], I16, kind="ExternalInput")
    dstf_d = nc.dram_tensor("dstf", [P, NCH8], F32, kind="ExternalInput")
    W1p = nc.dram_tensor("W1p", [F_IN, H], F32, kind="ExternalInput")
    W2p = nc.dram_tensor("W2p", [H, H], F32, kind="ExternalInput")
    W3p = nc.dram_tensor("W3p", [H, H], F32, kind="ExternalInput")
    Bbc_d = [nc.dram_tensor(f"B{l}bc", [P, GT * H], F32, kind="ExternalInput")
             for l in (1, 2, 3)]
    fw1_d = nc.dram_tensor("fw1", [H, HB], F32, kind="ExternalInput")
    fw2_d = nc.dram_tensor("fw2", [HB, C], F32, kind="ExternalInput")
    fb1bc = nc.dram_tensor("fb1bc", [P, HB], F32, kind="ExternalInput")
    fb2bc = nc.dram_tensor("fb2bc", [P, C], F32, kind="ExternalInput")
    cinv_d = nc.dram_tensor("cinv", [B, 1], F32, kind="ExternalInput")
    iota8_d = nc.dram_tensor("iota8", [P, 8 * P], F32, kind="ExternalInput")
    out_d = nc.dram_tensor("out", [B, C], F32, kind="ExternalOutput")

    ag_in = nc.dram_tensor("ag_in", [NSP, H], F32)
    table = nc.dram_tensor("table", [M * NSP, H], F32)
    pool_in = nc.dram_tensor("pool_in", [B, H], F32)
    pool_out = nc.dram_tensor("pool_out", [B, H], F32)

    groups = [list(range(M))]

    def super_rows(dram, g, width):
        rows = dram[g * GT * P:(g + 1) * GT * P, :]
        return rows.rearrange("(t p) j -> p t j", p=P)

    with tile.TileContext(nc) as tc:
        with (
            tc.tile_pool(name="resident", bufs=1) as rp,
            tc.tile_pool(name="work", bufs=3) as wp,
            tc.tile_pool(name="oh", bufs=6) as ohp,
            tc.tile_pool(name="gather", bufs=4) as gp,
            tc.tile_pool(name="pseg", bufs=2, space="PSUM") as pseg,
            tc.tile_pool(name="ptr", bufs=1, space="PSUM") as ptr,
            tc.tile_pool(name="pu", bufs=1, space="PSUM") as pu,
            tc.tile_pool(name="pacc", bufs=1, space="PSUM") as pacc,
        ):
            ident = rp.tile([P, P], F32, tag="ident")
            make_identity(nc, ident[:])
            nc.gpsimd.load_library(library_config.mlp)

            iota8 = rp.tile([P, 8 * P], F32, tag="iota8")
            nc.sync.dma_start(out=iota8[:], in_=iota8_d[:, :])
            dstf = rp.tile([P, NCH8], F32, tag="dstf")
            nc.sync.dma_start(out=dstf[:], in_=dstf_d[:, :])
            gidx = rp.tile([P, IC], I16, tag="gidx")
            nc.sync.dma_start(out=gidx[:], in_=gidx_d[:, :])
            batchf = rp.tile([P, T], F32, tag="batchf")
            nc.sync.dma_start(out=batchf[:], in_=batch_t_d[:, :])

            ones_t = rp.tile([P, H], F32, tag="ones")
            nc.vector.memset(ones_t[:], 1.0)
            dinv_ts = rp.tile([P, T], F32, tag="dinv_ts")
            nc.sync.dma_start(out=dinv_ts[:], in_=dinv_t_d[:, :])
            dinvbc = rp.tile([P, T * H], F32, tag="dinvbc")
            for t in range(T):
                nc.vector.tensor_scalar(
                    dinvbc[:, t * H:(t + 1) * H], ones_t[:],
                    dinv_ts[:, t:t + 1], None, ALU.mult)

            uloc = rp.tile([P, T * H], F32, tag="uloc")
            acc = rp.tile([P, T * H], F32, tag="acc")

            w1s = rp.tile([F_IN, H], F32, tag="w1s")
            nc.sync.dma_start(out=w1s[:], in_=W1p[:, :])
            w2s = rp.tile([H, H], F32, tag="w2s")
            nc.sync.dma_start(out=w2s[:], in_=W2p[:, :])
            w3s = rp.tile([H, H], F32, tag="w3s")
            nc.sync.dma_start(out=w3s[:], in_=W3p[:, :])
            bbc = []
            for l in range(3):
                t_ = rp.tile([P, GT * H], F32, tag=f"bbc{l}")
                nc.sync.dma_start(out=t_[:], in_=Bbc_d[l][:, :])
                bbc.append(t_)
            fw1s = rp.tile([H, HB], F32, tag="fw1s")
            nc.sync.dma_start(out=fw1s[:], in_=fw1_d[:, :])
            fw2s = rp.tile([HB, C], F32, tag="fw2s")
            nc.sync.dma_start(out=fw2s[:], in_=fw2_d[:, :])
            fb1s = rp.tile([P, HB], F32, tag="fb1s")
            nc.sync.dma_start(out=fb1s[:], in_=fb1bc[:, :])
            fb2s = rp.tile([P, C], F32, tag="fb2s")
            nc.sync.dma_start(out=fb2s[:], in_=fb2bc[:, :])
            cinvs = rp.tile([B, 1], F32, tag="cinvs")
            nc.sync.dma_start(out=cinvs[:], in_=cinv_d[:, :])

            def dense_from(rt_ap, G, w_s, kdim):
                """u'[G] = dinv * (rt @ W'); write uloc[G] and ag_in[G]."""
                gslice = slice(G * GT * H, (G + 1) * GT * H)
                psT = ptr.tile([H, GT * P], F32, tag="psT")
                for t8 in range(GT):
                    nc.tensor.transpose(
                        out=psT[:kdim, t8 * P:(t8 + 1) * P],
                        in_=rt_ap[:, t8 * kdim:(t8 + 1) * kdim],
                        identity=ident[:])
                rtT = wp.tile([H, GT * P], F32, tag="rtT")
                nc.vector.tensor_copy(rtT[:kdim, :], psT[:kdim, :])
                psU = pu.tile([P, GT * H], F32, tag="psU")
                for t8 in range(GT):
                    nc.tensor.matmul(
                        out=psU[:, t8 * H:(t8 + 1) * H],
                        lhsT=rtT[:kdim, t8 * P:(t8 + 1) * P],
                        rhs=w_s[:], start=True, stop=True)
                nc.vector.tensor_tensor(
                    out=uloc[:, gslice], in0=psU[:],
                    in1=dinvbc[:, gslice], op=ALU.mult)
                nc.sync.dma_start(
                    out=super_rows(ag_in, G, H),
                    in_=uloc[:, gslice].rearrange("p (t j) -> p t j", j=H))

            # ---------------- layer-0 dense: u1 = dinv * (x @ W1') --------
            for G in range(NG):
                xt = wp.tile([P, GT * F_IN], F32, tag="xt")
                nc.sync.dma_start(
                    out=xt[:].rearrange("p (t j) -> p t j", j=F_IN),
                    in_=super_rows(x_sh, G, F_IN))
                dense_from(xt[:], G, w1s, F_IN)

            oh_state = {"tile": None, "c0": -1}

            def get_oh(c):
                c0 = (c // 8) * 8
                if oh_state["c0"] != c0:
                    oh8 = ohp.tile([P, 8 * P], F32, tag="oh8")
                    nc.vector.tensor_tensor(
                        out=oh8[:].rearrange("p (c j) -> p c j", j=P),
                        in0=iota8[:].rearrange("p (c j) -> p c j", j=P),
                        in1=dstf[:, c0:c0 + 8].to_broadcast([P, 8, P]),
                        op=ALU.is_equal)
                    oh_state["tile"] = oh8
                    oh_state["c0"] = c0
                return oh_state["tile"][:, (c - c0) * P:(c - c0 + 1) * P]

            # ---------------- layers ----------------
            for l in range(3):
                nc.gpsimd.collective_compute(
                    "AllGather", ALU.bypass, replica_groups=groups,
                    ins=[ag_in.ap().opt()], outs=[table.ap().opt()])

                goff = 0     # rows consumed so far (global chunk counter)
                for q in range(NQ):
                    for G in range(NG):
                        rows_G = int(rows_qG[q, G])
                        if rows_G == 0:
                            continue
                        buf = gp.tile([P, (MAXR // P) * H], F32, tag="buf")
                        nc.gpsimd.dma_gather(
                            buf[:, :(rows_G // P) * H].rearrange(
                                "p (c j) -> p c j", j=H),
                            table[q * QN:(q + 1) * QN, :],
                            gidx[:, goff // 16:(goff + rows_G) // 16],
                            rows_G, rows_G, H,
                            single_packet=SINGLE_PACKET)
                        seg = pseg.tile([P, GT * H], F32, tag="seg")
                        lc = 0
                        for t8 in range(GT):
                            t = G * GT + t8
                            k = int(k_qt[q, t])
                            for j in range(k):
                                c = goff // P + lc
                                oh = get_oh(c)
                                nc.tensor.matmul(
                                    out=seg[:, t8 * H:(t8 + 1) * H],
                                    lhsT=oh,
                                    rhs=buf[:, lc * H:(lc + 1) * H],
                                    start=(j == 0), stop=(j == k - 1))
                                lc += 1
                        gslice = slice(G * GT * H, (G + 1) * GT * H)
                        if q == 0:
                            nc.vector.tensor_copy(acc[:, gslice], seg[:])
                        else:
                            nc.vector.tensor_add(acc[:, gslice],
                                                 acc[:, gslice], seg[:])
                        goff += rows_G

                # tile phase: rt -> (dense | pool)
                if l == 2:
                    pool_ps = pacc.tile([B, H], F32, tag="pool_ps")
                for G in range(NG):
                    gslice = slice(G * GT * H, (G + 1) * GT * H)
                    rt = wp.tile([P, GT * H], F32, tag="rt")
                    nc.vector.tensor_add(rt[:], acc[:, gslice],
                                         uloc[:, gslice])
                    nc.vector.tensor_tensor(out=rt[:], in0=rt[:],
                                            in1=dinvbc[:, gslice],
                                            op=ALU.mult)
                    nc.vector.tensor_add(rt[:], rt[:], bbc[l][:])
                    nc.vector.tensor_scalar_max(rt[:], rt[:], 0.0)
                    if l < 2:
                        dense_from(rt[:], G, w2s if l == 0 else w3s, H)
                    else:
                        for t8 in range(GT):
                            t = G * GT + t8
                            S = wp.tile([P, P], F32, tag="S")
                            nc.vector.tensor_scalar(
                                S[:], iota8[:, :P], batchf[:, t:t + 1],
                                None, ALU.is_equal)
                            nc.tensor.matmul(
                                out=pool_ps[:], lhsT=S[:],
                                rhs=rt[:, t8 * H:(t8 + 1) * H],
                                start=(t == 0), stop=(t == T - 1))

            # ---------------- pool + MLP head ----------------
            pool_sb = wp.tile([B, H], F32, tag="pool_sb")
            nc.vector.tensor_copy(pool_sb[:], pool_ps[:])
            nc.sync.dma_start(out=pool_in[:, :], in_=pool_sb[:])
            nc.gpsimd.collective_compute(
                "AllReduce", ALU.add, replica_groups=groups,
                ins=[pool_in.ap().opt()], outs=[pool_out.ap().opt()])

            pooled = wp.tile([B, H], F32, tag="pooled")
            nc.sync.dma_start(out=pooled[:], in_=pool_out[:, :])
            nc.vector.tensor_scalar(pooled[:], pooled[:], cinvs[:], None,
                                    ALU.mult)

            trp = pu.tile([H, B], F32, tag="psU")
            nc.tensor.transpose(out=trp[:], in_=pooled[:], identity=ident[:])
            pT = wp.tile([H, B], F32, tag="pT")
            nc.vector.tensor_copy(pT[:], trp[:])
            h1ps = pu.tile([B, HB], F32, tag="psU")
            nc.tensor.matmul(out=h1ps[:], lhsT=pT[:], rhs=fw1s[:],
                             start=True, stop=True)
            h1 = wp.tile([B, HB], F32, tag="h1")
            nc.vector.tensor_add(h1[:], h1ps[:], fb1s[:])
            nc.vector.tensor_scalar_max(h1[:], h1[:], 0.0)
            tr2 = pu.tile([HB, B], F32, tag="psU")
            nc.tensor.transpose(out=tr2[:], in_=h1[:], identity=ident[:])
            h1T = wp.tile([HB, B], F32, tag="h1T")
            nc.vector.tensor_copy(h1T[:], tr2[:])
            o_ps = pu.tile([B, C], F32, tag="psU")
            nc.tensor.matmul(out=o_ps[:], lhsT=h1T[:], rhs=fw2s[:],
                             start=True, stop=True)
            o_sb = wp.tile([B, C], F32, tag="o_sb")
            nc.vector.tensor_add(o_sb[:], o_ps[:], fb2s[:])
            nc.sync.dma_start(out=out_d[:, :], in_=o_sb[:])

    nc.compile()
    return nc


_CACHE: dict = {}


def kernel(**inputs) -> np.ndarray:
    global LAST_EXEC_NS
    in_maps, meta = preprocess(**inputs)
    key = cache_key(meta)
    nc = _CACHE.get(key)
    if nc is None:
        nc = build_program(meta)
        _CACHE[key] = nc
    res = bass_utils.run_bass_kernel_spmd(
        nc, in_maps, core_ids=list(range(M)), trace=TRACE)
    LAST_EXEC_NS = res.exec_time_ns
    return np.asarray(res.results[0]["out"])
